# revision 1
# baseline (speedup 1.0000x reference)
"""Trainium2 Bass kernel: 5-tap Kaiser circular filter along H then W of a
(16, 3, 1024, 1024) fp32 tensor. Data-parallel over batch across 8 cores.

Per core: 2 batches x 3 channels = 6 independent (1024, 1024) slices.

Per slice (instruction-count-optimized, slice-wide tiles):
  - in_big [128, 9x1024]: chunk j holds input rows [124j-2, 124j+126) mod
    1024 (2-row halos; 5 strided DMAs cover interior + wraps).
  - H-filter: banded matmul per chunk, stationary A[k, m] = kernel[m+4-k]
    ([128, 124], same for all chunks), 2 x N=512 fp32 matmuls -> PSUM
    [124, 1024].
  - ACT copies PSUM -> ext_big [124, 9x1028] (chunk = 2-col halo | 1024 |
    2-col halo); two slice-wide 3D copies fill all W halos.
  - W-filter: 5 slice-wide DVE multiply-accumulates over [124, 9, 1024]
    with free-dim shifts.
  - 2 DMAs store out_big back to HBM.
"""

import numpy as np

B, C, H, W = 16, 3, 1024, 1024
N_CORES = 8
S = (B // N_CORES) * C  # slices per core
TAPS = 5
HALO = TAPS // 2  # 2
STRIDE = 124  # output rows per block
NBLK = 9  # ceil(1024 / 124); last block keeps only 32 of 124 rows
TAIL_M = H - 8 * STRIDE  # 32
CW = W + 2 * HALO  # ext chunk width 1028

_cache = {}


def _build_with_taps(kk, repeat=1, io_external=True, zero_x=False, stages="full", jsplit=9):
    """kk: numpy [5] float32 tap weights. Returns compiled Bass object."""
    import concourse.bass as bass
    import concourse.bacc as bacc
    import concourse.mybir as mybir
    import concourse.tile as tile

    f32 = mybir.dt.float32
    nc = bacc.Bacc("TRN2", target_bir_lowering=False, debug=False, num_devices=N_CORES)

    if io_external:
        x_d = nc.dram_tensor("x", [S, H, W], f32, kind="ExternalInput")
        y_d = nc.dram_tensor("y", [S, H, W], f32, kind="ExternalOutput")
    else:
        x_d = nc.dram_tensor("x", [S, H, W], f32)
        y_d = nc.dram_tensor("y", [S, H, W], f32)
    a_d = nc.dram_tensor("afilt", [128, STRIDE], f32, kind="ExternalInput")

    kk = [float(v) for v in kk]

    def xap(s, row0, dims):
        """Raw strided AP into x_d at slice s, starting at row row0.

        dims: list of [row_step_rows, count] pairs followed by the implicit
        innermost [1, W] element run.
        """
        off = (s * H + row0) * W
        ap = [[st * W, ct] for st, ct in dims] + [[1, W]]
        return bass.AP(x_d, off, ap)

    def yap(s, row0, dims):
        off = (s * H + row0) * W
        ap = [[st * W, ct] for st, ct in dims] + [[1, W]]
        return bass.AP(y_d, off, ap)

    with tile.TileContext(nc) as tc:
        with (
            tc.tile_pool(name="wpool", bufs=1) as wpool,
            tc.tile_pool(name="inp", bufs=2) as inp,
            tc.tile_pool(name="psum", bufs=2, space="PSUM") as psum,
            tc.tile_pool(name="extp", bufs=1) as extp,
            tc.tile_pool(name="outp", bufs=2) as outp,
        ):
            a_s = wpool.tile([128, STRIDE], f32)
            nc.sync.dma_start(a_s[:], a_d[:])

            if zero_x:
                zt = wpool.tile([128, W], f32)
                nc.gpsimd.memset(zt[:], 0.0)
                for s in range(S):
                    for r in range(0, H, 128):
                        nc.sync.dma_start(x_d[s, r : r + 128, :], zt[:])

            for _ in range(repeat):
                for s in range(S):
                    in_big = inp.tile([128, NBLK * W], f32)
                    in3 = in_big.rearrange("p (j w) -> p j w", w=W)

                    # chunk j, partition k <- row (124j - 2 + k) mod 1024
                    # interior: k=2..127 for j=0..7
                    nc.sync.dma_start(
                        in3[2:128, 0:8, :], xap(s, 0, [[1, 126], [STRIDE, 8]])
                    )
                    # k=0..1 for j=1..7 (rows 124j-2, 124j-1)
                    nc.sync.dma_start(
                        in3[0:2, 1:8, :], xap(s, STRIDE - 2, [[1, 2], [STRIDE, 7]])
                    )
                    # j=0, k=0..1 <- rows 1022, 1023 (top wrap)
                    nc.sync.dma_start(in3[0:2, 0, :], xap(s, H - 2, [[1, 2]]))
                    # j=8: k=0..33 <- rows 990..1023, k=34..127 <- rows 0..93
                    nc.sync.dma_start(
                        in3[0:34, 8, :], xap(s, 8 * STRIDE - 2, [[1, 34]])
                    )
                    nc.sync.dma_start(in3[34:128, 8, :], xap(s, 0, [[1, 94]]))

                    if stages == "dma":
                        nc.sync.dma_start(
                            yap(s, 0, [[1, STRIDE], [STRIDE, 8]]),
                            in3[0:STRIDE, 0:8, :],
                        )
                        nc.sync.dma_start(
                            yap(s, 8 * STRIDE, [[1, TAIL_M]]), in3[0:TAIL_M, 8, :]
                        )
                        continue

                    ext = extp.tile([STRIDE, NBLK * CW], f32)
                    ext3 = ext.rearrange("p (j c) -> p j c", c=CW)

                    # pairs of blocks share one 4-bank PSUM tile -> one ACT copy
                    for j0 in range(0, NBLK, 2):
                        npair = min(2, NBLK - j0)
                        ps = psum.tile([STRIDE, 2 * W], f32)
                        for b in range(npair):
                            for half in range(0, W, 512):
                                nc.tensor.matmul(
                                    ps[:, b * W + half : b * W + half + 512],
                                    a_s[:],
                                    in3[:, j0 + b, half : half + 512],
                                    start=True,
                                    stop=True,
                                )
                        ps3 = ps.rearrange("p (b w) -> p b w", w=W)
                        nc.scalar.copy(
                            ext3[:, j0 : j0 + npair, HALO : HALO + W],
                            ps3[:, 0:npair, :],
                        )

                    if stages == "mm":
                        nc.sync.dma_start(
                            yap(s, 0, [[1, STRIDE], [STRIDE, 8]]),
                            ext3[:, 0:8, HALO : HALO + W],
                        )
                        nc.sync.dma_start(
                            yap(s, 8 * STRIDE, [[1, TAIL_M]]),
                            ext3[0:TAIL_M, 8, HALO : HALO + W],
                        )
                        continue

                    # circular W halos, all chunks at once
                    nc.scalar.copy(ext3[:, :, 0:HALO], ext3[:, :, W : W + HALO])
                    nc.scalar.copy(
                        ext3[:, :, HALO + W : CW], ext3[:, :, HALO : 2 * HALO]
                    )

                    out_big = outp.tile([STRIDE, NBLK * W], f32)
                    out3 = out_big.rearrange("p (j w) -> p j w", w=W)

                    # out[p, j, w] = sum_d kk[d] * ext3[p, j, w + 4 - d]
                    # W-pass split: DVE (fused MACs) chunks [0, JSPLIT),
                    # GPSIMD (mul + add pairs; Pool lacks TensorScalarPtr)
                    # chunks [JSPLIT, NBLK).
                    JSPLIT = jsplit
                    nc.vector.tensor_scalar_mul(
                        out3[:, 0:JSPLIT, :], ext3[:, 0:JSPLIT, 4 : 4 + W], kk[0]
                    )
                    for d in range(1, TAPS):
                        sft = 4 - d
                        nc.vector.scalar_tensor_tensor(
                            out3[:, 0:JSPLIT, :],
                            ext3[:, 0:JSPLIT, sft : sft + W],
                            kk[d],
                            out3[:, 0:JSPLIT, :],
                            mybir.AluOpType.mult,
                            mybir.AluOpType.add,
                        )
                    if JSPLIT < NBLK:
                        gtmp = outp.tile([STRIDE, (NBLK - JSPLIT) * W], f32)
                        g3 = gtmp.rearrange("p (j w) -> p j w", w=W)
                        nc.gpsimd.tensor_scalar_mul(
                            out3[:, JSPLIT:NBLK, :],
                            ext3[:, JSPLIT:NBLK, 4 : 4 + W],
                            kk[0],
                        )
                        for d in range(1, TAPS):
                            sft = 4 - d
                            nc.gpsimd.tensor_scalar_mul(
                                g3[:, :, :],
                                ext3[:, JSPLIT:NBLK, sft : sft + W],
                                kk[d],
                            )
                            nc.gpsimd.tensor_add(
                                out3[:, JSPLIT:NBLK, :],
                                out3[:, JSPLIT:NBLK, :],
                                g3[:, :, :],
                            )

                    # store: blocks 0..7 keep all 124 rows, block 8 keeps 32
                    nc.sync.dma_start(
                        yap(s, 0, [[1, STRIDE], [STRIDE, 8]]),
                        out3[:, 0:8, :],
                    )
                    nc.sync.dma_start(
                        yap(s, 8 * STRIDE, [[1, TAIL_M]]), out3[0:TAIL_M, 8, :]
                    )

    nc.compile()
    return nc


def _afilt_from_taps(kk):
    a = np.zeros((128, STRIDE), dtype=np.float32)
    for mcol in range(STRIDE):
        for d in range(TAPS):
            k = mcol + 4 - d
            if 0 <= k < 128:
                a[k, mcol] = kk[d]
    return a


def kernel(x, kernel):
    from concourse.bass_utils import run_bass_kernel_spmd

    x = np.asarray(x, dtype=np.float32)
    kk = np.asarray(kernel, dtype=np.float32).reshape(-1)
    assert x.shape == (B, C, H, W)
    assert kk.shape == (TAPS,)

    key = kk.tobytes()
    if key not in _cache:
        _cache[key] = _build_with_taps(kk)
    nc = _cache[key]

    afilt = _afilt_from_taps(kk)
    per_core = B // N_CORES
    in_maps = []
    for i in range(N_CORES):
        shard = np.ascontiguousarray(
            x[i * per_core : (i + 1) * per_core].reshape(S, H, W)
        )
        in_maps.append({"x": shard, "afilt": afilt})

    res = run_bass_kernel_spmd(nc, in_maps, core_ids=list(range(N_CORES)))
    out = np.empty((B, C, H, W), dtype=np.float32)
    for i in range(N_CORES):
        out[i * per_core : (i + 1) * per_core] = res.results[i]["y"].reshape(
            per_core, C, H, W
        )
    return out



# revision 2
# speedup vs baseline: 107.6172x; 107.6172x over previous
"""Trainium2 Bass kernel: 5-tap Kaiser circular filter along H and W of a
(16, 3, 1024, 1024) fp32 tensor. Data-parallel over batch across 8 cores.

Per core: 2 batches x 3 channels = 6 independent (1024, 1024) slices.

Structure (driven by measured per-dma_start HWDGE ring occupancy of
~10-13 us, serial per ring):
  - Host pads each slice circularly to x_pad [1120, 1028] (2-row/col halos
    plus 94 wrap rows) so the whole 9-chunk banded layout loads with ONE
    DMA per slice (sync/SP HWDGE ring).
  - Fused separable filter: out = sum_d k[d] * (A_H^T @ x shifted d in W),
    i.e. 5 accumulating float32r matmuls per 512-col PSUM block with
    lhsT = k[d] * A_H (the 5-band H circulant band, [128, 124]) and
    free-dim-shifted rhs slices. No vector-engine W-pass at all.
  - PSUM -> SBUF evacuation on the vector engine; ONE store DMA per slice
    (scalar/ACT HWDGE ring) writes rows 0..1115 of y_pad [1116, 1024]
    (rows >= 1024 are wrapped duplicates); host slices [:1024].
"""

import numpy as np

B, C, H, W = 16, 3, 1024, 1024
N_CORES = 8
S = (B // N_CORES) * C  # slices per core
TAPS = 5
HALO = TAPS // 2  # 2
STRIDE = 124  # output rows per block
NBLK = 9  # ceil(1024 / 124)
TAIL_M = H - 8 * STRIDE  # 32
CW = W + 2 * HALO  # padded slice width 1028
PH = 2 + H + (128 - 2 - TAIL_M)  # padded rows: 1120 (covers rows -2..1117)
OH = 8 * STRIDE + 124  # stored output rows: 1116 (rows >= 1024 are junk)

_cache = {}


def _build_with_taps(kk, repeat=1):
    """kk: numpy [5] float32 tap weights. Returns compiled Bass object."""
    import concourse.bass as bass
    import concourse.bacc as bacc
    import concourse.mybir as mybir
    import concourse.tile as tile

    f32 = mybir.dt.float32
    f32r = mybir.dt.float32r
    nc = bacc.Bacc("TRN2", target_bir_lowering=False, debug=False, num_devices=N_CORES)

    x_d = nc.dram_tensor("xpad", [S, PH, CW], f32r, kind="ExternalInput")
    y_d = nc.dram_tensor("ypad", [S, OH, W], f32, kind="ExternalOutput")
    a_d = nc.dram_tensor("afilt5", [128, TAPS * STRIDE], f32r, kind="ExternalInput")

    with tile.TileContext(nc) as tc:
        with (
            tc.tile_pool(name="wpool", bufs=1) as wpool,
            tc.tile_pool(name="inp", bufs=2) as inp,
            tc.tile_pool(name="psum", bufs=2, space="PSUM") as psum,
            tc.tile_pool(name="outp", bufs=2) as outp,
        ):
            a_s = wpool.tile([128, TAPS * STRIDE], f32r)
            nc.sync.dma_start(a_s[:], a_d[:])
            a3 = a_s.rearrange("p (d m) -> p d m", m=STRIDE)

            for _ in range(repeat):
                for s in range(S):
                    in_big = inp.tile([128, NBLK * CW], f32r)
                    in3 = in_big.rearrange("p (j c) -> p j c", c=CW)

                    # ONE load: in3[k, j, c] = x_pad[s, 124j + k, c]
                    nc.sync.dma_start(
                        in3[:, :, :],
                        bass.AP(
                            x_d,
                            s * PH * CW,
                            [[CW, 128], [STRIDE * CW, NBLK], [1, CW]],
                        ),
                    )

                    out_big = outp.tile([STRIDE, NBLK * W], f32)
                    out3 = out_big.rearrange("p (j w) -> p j w", w=W)

                    # pairs of blocks share one 4-bank PSUM tile
                    for j0 in range(0, NBLK, 2):
                        npair = min(2, NBLK - j0)
                        ps = psum.tile([STRIDE, 2 * W], f32)
                        for b in range(npair):
                            for half in range(0, W, 512):
                                for d in range(TAPS):
                                    nc.tensor.matmul(
                                        ps[:, b * W + half : b * W + half + 512],
                                        a3[:, d, :],
                                        in3[:, j0 + b, half + d : half + d + 512],
                                        start=(d == 0),
                                        stop=(d == TAPS - 1),
                                    )
                        ps3 = ps.rearrange("p (b w) -> p b w", w=W)
                        nc.vector.tensor_copy(
                            out3[:, j0 : j0 + npair, :], ps3[:, 0:npair, :]
                        )

                    # ONE store: y_pad[s, 124j + m, :] = out3[m, j, :]
                    nc.scalar.dma_start(
                        bass.AP(
                            y_d,
                            s * OH * W,
                            [[W, STRIDE], [STRIDE * W, NBLK], [1, W]],
                        ),
                        out3[:, :, :],
                    )

    nc.compile()
    return nc


def _afilt_from_taps(kk):
    """[128, 5*124]: a5[k, d*124+m] = kk[d] * kk[dh] at k = m + 4 - dh."""
    a = np.zeros((128, TAPS * STRIDE), dtype=np.float32)
    for mcol in range(STRIDE):
        for dh in range(TAPS):
            k = mcol + 4 - dh
            if 0 <= k < 128:
                for d in range(TAPS):
                    a[k, d * STRIDE + mcol] = kk[d] * kk[dh]
    return a


def _pad_shard(shard):
    """[S, H, W] -> circularly padded [S, PH, CW]."""
    return np.ascontiguousarray(
        np.pad(
            shard,
            ((0, 0), (HALO, PH - H - HALO), (HALO, CW - W - HALO)),
            mode="wrap",
        )
    )


def make_in_maps(x, kk):
    afilt = _afilt_from_taps(kk)
    per_core = B // N_CORES
    in_maps = []
    for i in range(N_CORES):
        shard = x[i * per_core : (i + 1) * per_core].reshape(S, H, W)
        in_maps.append({"xpad": _pad_shard(shard), "afilt5": afilt})
    return in_maps


def kernel(x, kernel):
    from concourse.bass_utils import run_bass_kernel_spmd

    x = np.asarray(x, dtype=np.float32)
    kk = np.asarray(kernel, dtype=np.float32).reshape(-1)
    assert x.shape == (B, C, H, W)
    assert kk.shape == (TAPS,)

    key = kk.tobytes()
    if key not in _cache:
        _cache[key] = _build_with_taps(kk)
    nc = _cache[key]

    in_maps = make_in_maps(x, kk)
    res = run_bass_kernel_spmd(nc, in_maps, core_ids=list(range(N_CORES)))
    per_core = B // N_CORES
    out = np.empty((B, C, H, W), dtype=np.float32)
    for i in range(N_CORES):
        out[i * per_core : (i + 1) * per_core] = res.results[i]["ypad"][
            :, :H, :
        ].reshape(per_core, C, H, W)
    return out
